# revision 12
# baseline (speedup 1.0000x reference)
"""Multi-head self-attention (CogView PB-relax variant) on 8 TRN2 NeuronCores.

Problem: B=2, S=2048, D=1024, H=16 heads, Dh=64.
  q/k/v = hidden @ W{q,k,v}.T + b          (per-head slices)
  scores = (q k^T + attn_bias) / 8 + (1-mask)*(-BIG)
  out    = softmax(scores) @ v             (PB-relax softmax == plain softmax)

Sharding: tensor-parallel over heads. Core c owns heads (2c, 2c+1) for both
batch rows: it reads full hidden, W-row slices [128c:128c+128], bias slice
[h=2c:2c+2], and writes output channels [128c:128(c+1)].

v2 design notes (from v1 profiling):
  - PE-transposes (LDWEIGHTS per 128x128 block + cold-clock streams) were the
    bottleneck; ALL data transposes now ride the 2-byte DMA xbar on bf16:
    one dma_start(transpose=True) turns a [128, N] bf16 SBUF tile into
    N/128 transposed blocks ([128, N/128, 128] out AP).
  - everything PE touches is bf16 (1 cyc/row, FWL weight loads, HAM stays
    warm since the transpose-mode ops are gone).
  - scores computed transposed, tile [k=128, q=512] PSUM: bias^T injected by
    an identity matmul (start=True), then k q^T accumulates (contraction 64,
    both heads concurrently via tile_position row groups).
  - exp on ACT: out = exp(in*0.125 + maskbias[k]) with maskbias a
    per-partition column — the attention mask is free.
  - AV: lhsT = [v | 1] (65 cols), so ctx^T row 64 = masked softmax
    denominator. Epilogue transposes back via PE (small, f32 exact) and
    divides with per-partition reciprocals.
"""

import numpy as np

import concourse.bass as bass
import concourse.mybir as mybir
import concourse.tile as tile
from concourse import bacc, bass_utils
from concourse.masks import make_identity

F32 = mybir.dt.float32
BF16 = mybir.dt.bfloat16
I32 = mybir.dt.int32
Exp = mybir.ActivationFunctionType.Exp

B, S, D = 2, 2048, 1024
NCORES = 8
HPC = 2            # heads per core
OC = HPC * 64      # 128 output channels per core
QB = 512           # q block (free dim of score tiles)
NQB = S // QB      # 4
NKC = S // 128     # 16 k-chunks per batch row
NSB = (B * S) // 512   # 8 token blocks for projections
NDC = D // 128     # 8 contraction chunks

MASK_NEG = -30000.0
SCALE = 0.125


def _build_program():
    nc = bacc.Bacc(
        "TRN2", target_bir_lowering=False, debug=False, num_devices=NCORES
    )
    hidden = nc.dram_tensor("hidden_state", [B, S, D], F32, kind="ExternalInput").ap()
    amask = nc.dram_tensor("attention_mask", [B, S], I32, kind="ExternalInput").ap()
    abias = nc.dram_tensor("attention_bias", [HPC, S, S], F32, kind="ExternalInput").ap()
    wq = nc.dram_tensor("wq", [OC, D], F32, kind="ExternalInput").ap()
    bq = nc.dram_tensor("bq", [OC], F32, kind="ExternalInput").ap()
    wk = nc.dram_tensor("wk", [OC, D], F32, kind="ExternalInput").ap()
    bk = nc.dram_tensor("bk", [OC], F32, kind="ExternalInput").ap()
    wv = nc.dram_tensor("wv", [OC, D], F32, kind="ExternalInput").ap()
    bv = nc.dram_tensor("bv", [OC], F32, kind="ExternalInput").ap()
    out = nc.dram_tensor("out", [B, S, OC], F32, kind="ExternalOutput").ap()

    with tile.TileContext(nc) as tc:
        _attention(tc, out, hidden, amask, abias,
                   [wq, wk, wv], [bq, bk, bv])

    nc.compile()
    return nc


def _attention(tc, out, hidden, amask, abias, ws, bs):
    nc = tc.nc
    hflat = hidden.flatten_outer_dims()          # [4096, 1024]

    with tc.tile_pool(name="singles", bufs=1) as singles:
        ident = singles.tile([128, 128], F32)    # for epilogue PE transposes
        make_identity(nc, ident)
        identb = singles.tile([128, 128], BF16)  # for bias-inject matmuls
        make_identity(nc, identb)

        # --- mask -> additive bias column layout [128, B, NKC] ------------
        mi = singles.tile([128, B, NKC], I32)
        nc.sync.dma_start(out=mi, in_=amask.rearrange("b (c p) -> p b c", p=128))
        mf = singles.tile([128, B, NKC], F32)
        nc.vector.tensor_copy(out=mf, in_=mi)
        mb = singles.tile([128, B, NKC], F32)
        nc.vector.tensor_scalar(
            out=mb, in0=mf, scalar1=-MASK_NEG, scalar2=MASK_NEG,
            op0=mybir.AluOpType.mult, op1=mybir.AluOpType.add,
        )

        # --- projection bias vectors [128, 1] -----------------------------
        bvec = []
        for i, b_ap in enumerate(bs):
            t = singles.tile([128, 1], F32, tag=f"bvec{i}")
            nc.sync.dma_start(out=t, in_=b_ap.rearrange("(p o) -> p o", o=1))
            bvec.append(t)

        ones_col = singles.tile([128, 1], BF16)
        nc.vector.memset(ones_col, 1.0)

        # --- W^T via xbar: wt3[w] = [d-local, dc, o] bf16 ------------------
        wt3 = []
        with tc.tile_pool(name="w_nat", bufs=2) as wnp:
            for i, w_ap in enumerate(ws):
                wn = wnp.tile([128, D], BF16)
                nc.gpsimd.dma_start(out=wn, in_=w_ap)   # cast f32->bf16
                t = singles.tile([128, NDC, 128], BF16, tag=f"wt{i}")
                nc.sync.dma_start(out=t, in_=wn, transpose=True)
                wt3.append(t)

        # --- persistent activations (bf16) --------------------------------
        qt2 = singles.tile([128, B * S], BF16, tag="qt2")
        kt2 = singles.tile([128, B * S], BF16, tag="kt2")
        va = singles.tile([128, 2 * NKC, 2 * 66], BF16, tag="va")

        # ============ phase 1: hidden^T + projections =====================
        with tc.tile_pool(name="h_nat", bufs=6) as hnp, \
             tc.tile_pool(name="h_t", bufs=2) as htp, \
             tc.tile_pool(name="v_t", bufs=3) as vtp, \
             tc.tile_pool(name="p_ps", bufs=4, space="PSUM") as pps:
            for sb in range(NSB):
                hts = htp.tile([128, NDC, 512], BF16)
                for i in range(4):
                    hn = hnp.tile([128, D], BF16)
                    nc.gpsimd.dma_start(
                        out=hn, in_=hflat[sb * 512 + i * 128:
                                          sb * 512 + (i + 1) * 128, :])
                    nc.sync.dma_start(
                        out=hts[:, :, i * 128:(i + 1) * 128], in_=hn,
                        transpose=True)
                for w in range(3):
                    pp = pps.tile([128, 512], F32)
                    for dc in range(NDC):
                        nc.tensor.matmul(
                            out=pp,
                            lhsT=wt3[w][:, dc, :],
                            rhs=hts[:, dc, :],
                            start=(dc == 0), stop=(dc == NDC - 1))
                    if w < 2:
                        dst = (qt2 if w == 0 else kt2)[:, sb * 512:(sb + 1) * 512]
                        nc.vector.tensor_scalar_add(
                            out=dst, in0=pp, scalar1=bvec[w])
                    else:
                        vt = vtp.tile([128, 512], BF16, name="vt")
                        nc.vector.tensor_scalar_add(out=vt, in0=pp, scalar1=bvec[2])
                        vts = vtp.tile([128, 4, 128], BF16, name="vts")
                        nc.sync.dma_start(out=vts, in_=vt, transpose=True)
                        for i in range(4):
                            kb = sb * 4 + i
                            for h in range(HPC):
                                nc.vector.tensor_copy(
                                    out=va[:, kb, h * 66:h * 66 + 64],
                                    in_=vts[:, i, h * 64:(h + 1) * 64])
                                nc.vector.tensor_copy(
                                    out=va[:, kb, h * 66 + 64:h * 66 + 65],
                                    in_=ones_col)

        # ============ phase 2: attention ==================================
        with tc.tile_pool(name="b_nat", bufs=4) as bnp, \
             tc.tile_pool(name="b_t", bufs=3) as btp, \
             tc.tile_pool(name="pt", bufs=6) as ptp, \
             tc.tile_pool(name="stage", bufs=3) as stp, \
             tc.tile_pool(name="osb", bufs=3) as osp, \
             tc.tile_pool(name="sc_ps", bufs=3, space="PSUM") as scp, \
             tc.tile_pool(name="ctx_ps", bufs=4, space="PSUM") as cxp:
            for qb in range(NQB):
                ctx = [[cxp.tile([65, QB], F32, tag="ctx", name=f"ctx{b}{h}")
                        for h in range(HPC)] for b in range(B)]
                # bias^T: cast-DMA 128 natural q-rows x 2048 k, xbar-transpose
                # into bT[h] = [k-local, q-sub, k-chunk, q-local] bf16
                bT = []
                for h in range(HPC):
                    t = btp.tile([128, 4, NKC, 128], BF16, tag="bT", name=f"bT{h}")
                    for i in range(4):
                        natq = bnp.tile([128, S], BF16, name="natq")
                        nc.gpsimd.dma_start(
                            out=natq,
                            in_=abias[h, qb * QB + i * 128:
                                      qb * QB + (i + 1) * 128, :])
                        nc.sync.dma_start(
                            out=t[:, i, :, :], in_=natq, transpose=True)
                    bT.append(t)
                for kc in range(NKC):
                    for b in range(B):
                        pts = []
                        for h in range(HPC):
                            sc = scp.tile([128, QB], F32, tag="sc", name="sc")
                            nc.tensor.matmul(
                                out=sc, lhsT=identb,
                                rhs=bT[h][:, :, kc, :],
                                start=True, stop=False,
                                skip_group_check=True)
                            nc.tensor.matmul(
                                out=sc,
                                lhsT=kt2[h * 64:(h + 1) * 64,
                                         b * S + kc * 128:
                                         b * S + (kc + 1) * 128],
                                rhs=qt2[h * 64:(h + 1) * 64,
                                        b * S + qb * QB:
                                        b * S + (qb + 1) * QB],
                                start=False, stop=True,
                                tile_position=(h * 64, 0),
                                skip_group_check=True)
                            pt = ptp.tile([128, QB], BF16, tag="pt", name="pt")
                            nc.scalar.activation(
                                out=pt, in_=sc, func=Exp,
                                bias=mb[:, b, kc:kc + 1], scale=SCALE)
                            pts.append(pt)
                        for h in range(HPC):
                            nc.tensor.matmul(
                                out=ctx[b][h],
                                lhsT=va[:, b * NKC + kc,
                                        h * 66:h * 66 + 65],
                                rhs=pts[h],
                                start=(kc == 0), stop=(kc == NKC - 1))
                # ---- epilogue: normalize, transpose to [q, d], store -----
                for b in range(B):
                    stage = stp.tile([128, QB], F32, tag="stage", name="stage")
                    rst = stp.tile([128, QB], F32, tag="rst", name="rst")
                    for h in range(HPC):
                        nc.vector.tensor_copy(
                            out=stage[h * 64:(h + 1) * 64, :],
                            in_=ctx[b][h][0:64, :])
                        # raw denominators at 32-aligned rows 0 / 32
                        nc.vector.tensor_copy(
                            out=rst[32 * h:32 * h + 1, :],
                            in_=ctx[b][h][64:65, :])
                    osb = osp.tile([128, 4, 128], F32, tag="osb", name="osb")
                    for i in range(4):
                        tp = scp.tile([128, 128], F32, tag="sc", name="ep_t")
                        rp = scp.tile([128, 128], F32, tag="sc", name="ep_r")
                        nc.tensor.transpose(
                            out=tp, in_=stage[:, i * 128:(i + 1) * 128],
                            identity=ident)
                        nc.tensor.transpose(
                            out=rp, in_=rst[:, i * 128:(i + 1) * 128],
                            identity=ident)
                        rcp = stp.tile([128, 2], F32, tag="rcp", name="rcp")
                        for h in range(HPC):
                            nc.vector.reciprocal(
                                out=rcp[:, h:h + 1],
                                in_=rp[:, 32 * h:32 * h + 1])
                            nc.vector.tensor_scalar_mul(
                                out=osb[:, i, h * 64:(h + 1) * 64],
                                in0=tp[:, h * 64:(h + 1) * 64],
                                scalar1=rcp[:, h:h + 1])
                    nc.sync.dma_start(
                        out=out[b, qb * QB:(qb + 1) * QB, :]
                        .rearrange("(i p) k -> p i k", p=128),
                        in_=osb)


_CACHE = {}


def _get_program():
    if "nc" not in _CACHE:
        _CACHE["nc"] = _build_program()
    return _CACHE["nc"]


def _shard_inputs(inputs):
    hs = np.ascontiguousarray(np.asarray(inputs["hidden_state"], dtype=np.float32))
    am = np.ascontiguousarray(np.asarray(inputs["attention_mask"], dtype=np.int32))
    ab = np.asarray(inputs["attention_bias"], dtype=np.float32)
    ws = {k: np.asarray(inputs[k], dtype=np.float32) for k in ("Wq", "Wk", "Wv")}
    vb = {k: np.asarray(inputs[k], dtype=np.float32) for k in ("bq", "bk", "bv")}
    in_maps = []
    for c in range(NCORES):
        r0, r1 = c * OC, (c + 1) * OC
        in_maps.append({
            "hidden_state": hs,
            "attention_mask": am,
            "attention_bias": np.ascontiguousarray(ab[0, HPC * c:HPC * (c + 1)]),
            "wq": np.ascontiguousarray(ws["Wq"][r0:r1]),
            "bq": np.ascontiguousarray(vb["bq"][r0:r1]),
            "wk": np.ascontiguousarray(ws["Wk"][r0:r1]),
            "bk": np.ascontiguousarray(vb["bk"][r0:r1]),
            "wv": np.ascontiguousarray(ws["Wv"][r0:r1]),
            "bv": np.ascontiguousarray(vb["bv"][r0:r1]),
        })
    return in_maps


def kernel(**inputs):
    nc = _get_program()
    in_maps = _shard_inputs(inputs)
    res = bass_utils.run_bass_kernel_spmd(
        nc, in_maps, core_ids=list(range(NCORES)))
    parts = [np.asarray(res.results[c]["out"]) for c in range(NCORES)]
    return np.concatenate(parts, axis=-1)


def run_profiled(inputs, trace=True):
    """test.py helper: returns (output, BassKernelResults)."""
    nc = _get_program()
    in_maps = _shard_inputs(inputs)
    res = bass_utils.run_bass_kernel_spmd(
        nc, in_maps, core_ids=list(range(NCORES)), trace=trace)
    parts = [np.asarray(res.results[c]["out"]) for c in range(NCORES)]
    return np.concatenate(parts, axis=-1), res


# revision 14
# speedup vs baseline: 1.0382x; 1.0382x over previous
"""Multi-head self-attention (CogView PB-relax variant) on 8 TRN2 NeuronCores.

Problem: B=2, S=2048, D=1024, H=16 heads, Dh=64.
  q/k/v = hidden @ W{q,k,v}.T + b          (per-head slices)
  scores = (q k^T + attn_bias) / 8 + (1-mask)*(-BIG)
  out    = softmax(scores) @ v             (PB-relax softmax == plain softmax)

Sharding: tensor-parallel over heads. Core c owns heads (2c, 2c+1) for both
batch rows: it reads full hidden, W-row slices [128c:128c+128], bias slice
[h=2c:2c+2], and writes output channels [128c:128(c+1)].

v2 design notes (from v1 profiling):
  - PE-transposes (LDWEIGHTS per 128x128 block + cold-clock streams) were the
    bottleneck; ALL data transposes now ride the 2-byte DMA xbar on bf16:
    one dma_start(transpose=True) turns a [128, N] bf16 SBUF tile into
    N/128 transposed blocks ([128, N/128, 128] out AP).
  - everything PE touches is bf16 (1 cyc/row, FWL weight loads, HAM stays
    warm since the transpose-mode ops are gone).
  - scores computed transposed, tile [k=128, q=512] PSUM: bias^T injected by
    an identity matmul (start=True), then k q^T accumulates (contraction 64,
    both heads concurrently via tile_position row groups).
  - exp on ACT: out = exp(in*0.125 + maskbias[k]) with maskbias a
    per-partition column — the attention mask is free.
  - AV: lhsT = [v | 1] (65 cols), so ctx^T row 64 = masked softmax
    denominator. Epilogue transposes back via PE (small, f32 exact) and
    divides with per-partition reciprocals.
"""

import numpy as np

import concourse.bass as bass
import concourse.mybir as mybir
import concourse.tile as tile
from concourse import bacc, bass_utils
from concourse.masks import make_identity

F32 = mybir.dt.float32
BF16 = mybir.dt.bfloat16
I32 = mybir.dt.int32
Exp = mybir.ActivationFunctionType.Exp

B, S, D = 2, 2048, 1024
NCORES = 8
HPC = 2            # heads per core
OC = HPC * 64      # 128 output channels per core
QB = 512           # q block (free dim of score tiles)
NQB = S // QB      # 4
NKC = S // 128     # 16 k-chunks per batch row
NSB = (B * S) // 512   # 8 token blocks for projections
NDC = D // 128     # 8 contraction chunks

MASK_NEG = -30000.0
SCALE = 0.125


def _build_program():
    nc = bacc.Bacc(
        "TRN2", target_bir_lowering=False, debug=False, num_devices=NCORES
    )
    hidden = nc.dram_tensor("hidden_state", [B, S, D], F32, kind="ExternalInput").ap()
    amask = nc.dram_tensor("attention_mask", [B, S], I32, kind="ExternalInput").ap()
    abias = nc.dram_tensor("attention_bias", [HPC, S, S], F32, kind="ExternalInput").ap()
    wq = nc.dram_tensor("wq", [OC, D], F32, kind="ExternalInput").ap()
    bq = nc.dram_tensor("bq", [OC], F32, kind="ExternalInput").ap()
    wk = nc.dram_tensor("wk", [OC, D], F32, kind="ExternalInput").ap()
    bk = nc.dram_tensor("bk", [OC], F32, kind="ExternalInput").ap()
    wv = nc.dram_tensor("wv", [OC, D], F32, kind="ExternalInput").ap()
    bv = nc.dram_tensor("bv", [OC], F32, kind="ExternalInput").ap()
    out = nc.dram_tensor("out", [B, S, OC], F32, kind="ExternalOutput").ap()

    with tile.TileContext(nc) as tc:
        _attention(tc, out, hidden, amask, abias,
                   [wq, wk, wv], [bq, bk, bv])

    nc.compile()
    return nc


def _attention(tc, out, hidden, amask, abias, ws, bs):
    nc = tc.nc
    hflat = hidden.flatten_outer_dims()          # [4096, 1024]

    with tc.tile_pool(name="singles", bufs=1) as singles:
        ident = singles.tile([128, 128], F32)    # for epilogue PE transposes
        make_identity(nc, ident)
        identb = singles.tile([128, 128], BF16)  # for bias-inject matmuls
        make_identity(nc, identb)

        # --- mask -> additive bias column layout [128, B, NKC] ------------
        mi = singles.tile([128, B, NKC], I32)
        nc.gpsimd.dma_start(out=mi, in_=amask.rearrange("b (c p) -> p b c", p=128))
        mf = singles.tile([128, B, NKC], F32)
        nc.vector.tensor_copy(out=mf, in_=mi)
        mb = singles.tile([128, B, NKC], F32)
        nc.vector.tensor_scalar(
            out=mb, in0=mf, scalar1=-MASK_NEG, scalar2=MASK_NEG,
            op0=mybir.AluOpType.mult, op1=mybir.AluOpType.add,
        )

        # --- projection bias vectors [128, 1] -----------------------------
        bvec = []
        for i, b_ap in enumerate(bs):
            t = singles.tile([128, 1], F32, tag=f"bvec{i}")
            nc.gpsimd.dma_start(out=t, in_=b_ap.rearrange("(p o) -> p o", o=1))
            bvec.append(t)

        ones_col = singles.tile([128, 1], BF16)
        nc.vector.memset(ones_col, 1.0)

        # --- W^T via xbar: wt3[w] = [d-local, dc, o] bf16 ------------------
        wt3 = []
        with tc.tile_pool(name="w_nat", bufs=2) as wnp:
            for i, w_ap in enumerate(ws):
                wn = wnp.tile([128, D], BF16)
                nc.gpsimd.dma_start(out=wn, in_=w_ap)   # cast f32->bf16
                t = singles.tile([128, NDC, 128], BF16, tag=f"wt{i}")
                nc.sync.dma_start(out=t, in_=wn, transpose=True)
                wt3.append(t)

        # --- persistent activations (bf16) --------------------------------
        qt2 = singles.tile([128, B * S], BF16, tag="qt2")
        kt2 = singles.tile([128, B * S], BF16, tag="kt2")
        va = singles.tile([128, 2 * NKC, 2 * 66], BF16, tag="va")

        # ============ phase 1: hidden^T + projections =====================
        with tc.tile_pool(name="h_nat", bufs=6) as hnp, \
             tc.tile_pool(name="h_t", bufs=2) as htp, \
             tc.tile_pool(name="v_t", bufs=3) as vtp, \
             tc.tile_pool(name="p_ps", bufs=4, space="PSUM") as pps:
            for sb in range(NSB):
                hts = htp.tile([128, NDC, 512], BF16)
                for i in range(4):
                    hn = hnp.tile([128, D], BF16)
                    nc.gpsimd.dma_start(
                        out=hn, in_=hflat[sb * 512 + i * 128:
                                          sb * 512 + (i + 1) * 128, :])
                    nc.sync.dma_start(
                        out=hts[:, :, i * 128:(i + 1) * 128], in_=hn,
                        transpose=True)
                for w in range(3):
                    pp = pps.tile([128, 512], F32)
                    for dc in range(NDC):
                        nc.tensor.matmul(
                            out=pp,
                            lhsT=wt3[w][:, dc, :],
                            rhs=hts[:, dc, :],
                            start=(dc == 0), stop=(dc == NDC - 1))
                    if w < 2:
                        dst = (qt2 if w == 0 else kt2)[:, sb * 512:(sb + 1) * 512]
                        nc.vector.tensor_scalar_add(
                            out=dst, in0=pp, scalar1=bvec[w])
                    else:
                        vt = vtp.tile([128, 512], BF16, name="vt")
                        nc.vector.tensor_scalar_add(out=vt, in0=pp, scalar1=bvec[2])
                        vts = vtp.tile([128, 4, 128], BF16, name="vts")
                        nc.sync.dma_start(out=vts, in_=vt, transpose=True)
                        for i in range(4):
                            kb = sb * 4 + i
                            for h in range(HPC):
                                nc.vector.tensor_copy(
                                    out=va[:, kb, h * 66:h * 66 + 64],
                                    in_=vts[:, i, h * 64:(h + 1) * 64])
                                nc.vector.tensor_copy(
                                    out=va[:, kb, h * 66 + 64:h * 66 + 65],
                                    in_=ones_col)

        # ============ phase 2: attention ==================================
        with tc.tile_pool(name="b_nat", bufs=6) as bnp, \
             tc.tile_pool(name="b_t", bufs=4) as btp, \
             tc.tile_pool(name="pt", bufs=6) as ptp, \
             tc.tile_pool(name="stage", bufs=3) as stp, \
             tc.tile_pool(name="osb", bufs=3) as osp, \
             tc.tile_pool(name="sc_ps", bufs=3, space="PSUM") as scp, \
             tc.tile_pool(name="ctx_ps", bufs=4, space="PSUM") as cxp:
            for qb in range(NQB):
                ctx = [[cxp.tile([65, QB], F32, tag="ctx", name=f"ctx{b}{h}")
                        for h in range(HPC)] for b in range(B)]
                # bias^T: cast-DMA 128 natural q-rows x 2048 k, xbar-transpose
                # into bT[h] = [k-local, q-sub, k-chunk, q-local] bf16
                bT = []
                for h in range(HPC):
                    t = btp.tile([128, 4, NKC, 128], BF16, tag="bT", name=f"bT{h}")
                    for i in range(4):
                        natq = bnp.tile([128, S], BF16, name="natq")
                        nc.gpsimd.dma_start(
                            out=natq,
                            in_=abias[h, qb * QB + i * 128:
                                      qb * QB + (i + 1) * 128, :])
                        nc.sync.dma_start(
                            out=t[:, i, :, :], in_=natq, transpose=True)
                    bT.append(t)
                for kc in range(NKC):
                    for b in range(B):
                        pts = []
                        for h in range(HPC):
                            sc = scp.tile([128, QB], F32, tag="sc", name="sc")
                            nc.tensor.matmul(
                                out=sc, lhsT=identb,
                                rhs=bT[h][:, :, kc, :],
                                start=True, stop=False,
                                skip_group_check=True)
                            nc.tensor.matmul(
                                out=sc,
                                lhsT=kt2[h * 64:(h + 1) * 64,
                                         b * S + kc * 128:
                                         b * S + (kc + 1) * 128],
                                rhs=qt2[h * 64:(h + 1) * 64,
                                        b * S + qb * QB:
                                        b * S + (qb + 1) * QB],
                                start=False, stop=True,
                                tile_position=(h * 64, 0),
                                skip_group_check=True)
                            pt = ptp.tile([128, QB], BF16, tag="pt", name="pt")
                            nc.scalar.activation(
                                out=pt, in_=sc, func=Exp,
                                bias=mb[:, b, kc:kc + 1], scale=SCALE)
                            pts.append(pt)
                        for h in range(HPC):
                            nc.tensor.matmul(
                                out=ctx[b][h],
                                lhsT=va[:, b * NKC + kc,
                                        h * 66:h * 66 + 65],
                                rhs=pts[h],
                                start=(kc == 0), stop=(kc == NKC - 1))
                # ---- epilogue: normalize, transpose to [q, d], store -----
                for b in range(B):
                    stage = stp.tile([128, QB], F32, tag="stage", name="stage")
                    rst = stp.tile([128, QB], F32, tag="rst", name="rst")
                    for h in range(HPC):
                        nc.vector.tensor_copy(
                            out=stage[h * 64:(h + 1) * 64, :],
                            in_=ctx[b][h][0:64, :])
                        # raw denominators at 32-aligned rows 0 / 32
                        nc.vector.tensor_copy(
                            out=rst[32 * h:32 * h + 1, :],
                            in_=ctx[b][h][64:65, :])
                    osb = osp.tile([128, 4, 128], F32, tag="osb", name="osb")
                    for i in range(4):
                        tp = scp.tile([128, 128], F32, tag="sc", name="ep_t")
                        rp = scp.tile([128, 128], F32, tag="sc", name="ep_r")
                        nc.tensor.transpose(
                            out=tp, in_=stage[:, i * 128:(i + 1) * 128],
                            identity=ident)
                        nc.tensor.transpose(
                            out=rp, in_=rst[:, i * 128:(i + 1) * 128],
                            identity=ident)
                        rcp = stp.tile([128, 2], F32, tag="rcp", name="rcp")
                        for h in range(HPC):
                            nc.vector.reciprocal(
                                out=rcp[:, h:h + 1],
                                in_=rp[:, 32 * h:32 * h + 1])
                            nc.vector.tensor_scalar_mul(
                                out=osb[:, i, h * 64:(h + 1) * 64],
                                in0=tp[:, h * 64:(h + 1) * 64],
                                scalar1=rcp[:, h:h + 1])
                    nc.gpsimd.dma_start(
                        out=out[b, qb * QB:(qb + 1) * QB, :]
                        .rearrange("(i p) k -> p i k", p=128),
                        in_=osb)


_CACHE = {}


def _get_program():
    if "nc" not in _CACHE:
        _CACHE["nc"] = _build_program()
    return _CACHE["nc"]


def _shard_inputs(inputs):
    hs = np.ascontiguousarray(np.asarray(inputs["hidden_state"], dtype=np.float32))
    am = np.ascontiguousarray(np.asarray(inputs["attention_mask"], dtype=np.int32))
    ab = np.asarray(inputs["attention_bias"], dtype=np.float32)
    ws = {k: np.asarray(inputs[k], dtype=np.float32) for k in ("Wq", "Wk", "Wv")}
    vb = {k: np.asarray(inputs[k], dtype=np.float32) for k in ("bq", "bk", "bv")}
    in_maps = []
    for c in range(NCORES):
        r0, r1 = c * OC, (c + 1) * OC
        in_maps.append({
            "hidden_state": hs,
            "attention_mask": am,
            "attention_bias": np.ascontiguousarray(ab[0, HPC * c:HPC * (c + 1)]),
            "wq": np.ascontiguousarray(ws["Wq"][r0:r1]),
            "bq": np.ascontiguousarray(vb["bq"][r0:r1]),
            "wk": np.ascontiguousarray(ws["Wk"][r0:r1]),
            "bk": np.ascontiguousarray(vb["bk"][r0:r1]),
            "wv": np.ascontiguousarray(ws["Wv"][r0:r1]),
            "bv": np.ascontiguousarray(vb["bv"][r0:r1]),
        })
    return in_maps


def kernel(**inputs):
    nc = _get_program()
    in_maps = _shard_inputs(inputs)
    res = bass_utils.run_bass_kernel_spmd(
        nc, in_maps, core_ids=list(range(NCORES)))
    parts = [np.asarray(res.results[c]["out"]) for c in range(NCORES)]
    return np.concatenate(parts, axis=-1)


def run_profiled(inputs, trace=True):
    """test.py helper: returns (output, BassKernelResults)."""
    nc = _get_program()
    in_maps = _shard_inputs(inputs)
    res = bass_utils.run_bass_kernel_spmd(
        nc, in_maps, core_ids=list(range(NCORES)), trace=trace)
    parts = [np.asarray(res.results[c]["out"]) for c in range(NCORES)]
    return np.concatenate(parts, axis=-1), res


# revision 17
# speedup vs baseline: 1.3810x; 1.3302x over previous
"""Multi-head self-attention (CogView PB-relax variant) on 8 TRN2 NeuronCores.

Problem: B=2, S=2048, D=1024, H=16 heads, Dh=64.
  q/k/v = hidden @ W{q,k,v}.T + b          (per-head slices)
  scores = (q k^T + attn_bias) / 8 + (1-mask)*(-BIG)
  out    = softmax(scores) @ v             (PB-relax softmax == plain softmax)

Sharding: tensor-parallel over heads. Core c owns heads (2c, 2c+1) for both
batch rows: it reads full hidden, W-row slices [128c:128c+128], bias slice
[h=2c:2c+2], and writes output channels [128c:128(c+1)].

v2 design notes (from v1 profiling):
  - PE-transposes (LDWEIGHTS per 128x128 block + cold-clock streams) were the
    bottleneck; ALL data transposes now ride the 2-byte DMA xbar on bf16:
    one dma_start(transpose=True) turns a [128, N] bf16 SBUF tile into
    N/128 transposed blocks ([128, N/128, 128] out AP).
  - everything PE touches is bf16 (1 cyc/row, FWL weight loads, HAM stays
    warm since the transpose-mode ops are gone).
  - scores computed transposed, tile [k=128, q=512] PSUM: bias^T injected by
    an identity matmul (start=True), then k q^T accumulates (contraction 64,
    both heads concurrently via tile_position row groups).
  - exp on ACT: out = exp(in*0.125 + maskbias[k]) with maskbias a
    per-partition column — the attention mask is free.
  - AV: lhsT = [v | 1] (65 cols), so ctx^T row 64 = masked softmax
    denominator. Epilogue transposes back via PE (small, f32 exact) and
    divides with per-partition reciprocals.
"""

import numpy as np

import concourse.bass as bass
import concourse.mybir as mybir
import concourse.tile as tile
from concourse import bacc, bass_utils
from concourse.masks import make_identity

F32 = mybir.dt.float32
BF16 = mybir.dt.bfloat16
I32 = mybir.dt.int32
Exp = mybir.ActivationFunctionType.Exp

B, S, D = 2, 2048, 1024
NCORES = 8
HPC = 2            # heads per core
OC = HPC * 64      # 128 output channels per core
QB = 512           # q block (free dim of score tiles)
NQB = S // QB      # 4
NKC = S // 128     # 16 k-chunks per batch row
NSB = (B * S) // 512   # 8 token blocks for projections
NDC = D // 128     # 8 contraction chunks

MASK_NEG = -30000.0
SCALE = 0.125


def _build_program():
    nc = bacc.Bacc(
        "TRN2", target_bir_lowering=False, debug=False, num_devices=NCORES
    )
    hidden = nc.dram_tensor("hidden_state", [B, S, D], F32, kind="ExternalInput").ap()
    amask = nc.dram_tensor("attention_mask", [B, S], I32, kind="ExternalInput").ap()
    abias = nc.dram_tensor("attention_bias", [HPC, S, S], F32, kind="ExternalInput").ap()
    wq = nc.dram_tensor("wq", [OC, D], F32, kind="ExternalInput").ap()
    bq = nc.dram_tensor("bq", [OC], F32, kind="ExternalInput").ap()
    wk = nc.dram_tensor("wk", [OC, D], F32, kind="ExternalInput").ap()
    bk = nc.dram_tensor("bk", [OC], F32, kind="ExternalInput").ap()
    wv = nc.dram_tensor("wv", [OC, D], F32, kind="ExternalInput").ap()
    bv = nc.dram_tensor("bv", [OC], F32, kind="ExternalInput").ap()
    out = nc.dram_tensor("out", [B, S, OC], F32, kind="ExternalOutput").ap()

    with tile.TileContext(nc) as tc:
        _attention(tc, out, hidden, amask, abias,
                   [wq, wk, wv], [bq, bk, bv])

    nc.compile()
    return nc


def _attention(tc, out, hidden, amask, abias, ws, bs):
    nc = tc.nc
    hflat = hidden.flatten_outer_dims()          # [4096, 1024]

    with tc.tile_pool(name="singles", bufs=1) as singles:
        ident = singles.tile([128, 128], F32)    # for epilogue PE transposes
        make_identity(nc, ident)
        identb = singles.tile([128, 128], BF16)  # for bias-inject matmuls
        make_identity(nc, identb)

        # --- mask -> additive bias column layout [128, B, NKC] ------------
        mi = singles.tile([128, B, NKC], I32)
        nc.gpsimd.dma_start(out=mi, in_=amask.rearrange("b (c p) -> p b c", p=128))
        mf = singles.tile([128, B, NKC], F32)
        nc.vector.tensor_copy(out=mf, in_=mi)
        mb = singles.tile([128, B, NKC], F32)
        nc.vector.tensor_scalar(
            out=mb, in0=mf, scalar1=-MASK_NEG, scalar2=MASK_NEG,
            op0=mybir.AluOpType.mult, op1=mybir.AluOpType.add,
        )

        # --- projection bias vectors [128, 1] -----------------------------
        bvec = []
        for i, b_ap in enumerate(bs):
            t = singles.tile([128, 1], F32, tag=f"bvec{i}")
            nc.gpsimd.dma_start(out=t, in_=b_ap.rearrange("(p o) -> p o", o=1))
            bvec.append(t)

        ones_col = singles.tile([128, 1], BF16)
        nc.vector.memset(ones_col, 1.0)

        # --- W^T via xbar: wt3[w] = [d-local, dc, o] bf16 ------------------
        wt3 = []
        with tc.tile_pool(name="w_nat", bufs=2) as wnp:
            for i, w_ap in enumerate(ws):
                wn = wnp.tile([128, D], BF16)
                nc.gpsimd.dma_start(out=wn, in_=w_ap)   # cast f32->bf16
                t = singles.tile([128, NDC, 128], BF16, tag=f"wt{i}")
                nc.sync.dma_start(out=t, in_=wn, transpose=True)
                wt3.append(t)

        # --- persistent activations (bf16) --------------------------------
        qt2 = singles.tile([128, B * S], BF16, tag="qt2")
        kt2 = singles.tile([128, B * S], BF16, tag="kt2")
        va = singles.tile([128, 2 * NKC, 2 * 66], BF16, tag="va")

        # bias prefetch pools live OUTSIDE the phase-1 scope so bias^T
        # streaming overlaps the projections (disjoint SBUF).
        bnp_cm = tc.tile_pool(name="b_nat", bufs=2)
        btp_cm = tc.tile_pool(name="b_t", bufs=3)
        bnp = bnp_cm.__enter__()
        btp = btp_cm.__enter__()

        def load_biasT(qb, h):
            natq = bnp.tile([128, 4, S], BF16, name="natq")
            nc.gpsimd.dma_start(
                out=natq,
                in_=abias[h, qb * QB:(qb + 1) * QB, :]
                .rearrange("(i p) k -> p i k", p=128))
            t = btp.tile([128, 4, NKC, 128], BF16, tag="bT", name=f"bT{h}")
            nc.sync.dma_start(
                out=t.rearrange("p i c x -> p (i c) x"),
                in_=natq.rearrange("p i k -> p (i k)"), transpose=True)
            return t

        # ============ phase 1: hidden^T + projections =====================
        with tc.tile_pool(name="h_nat", bufs=2) as hnp, \
             tc.tile_pool(name="h_t", bufs=2) as htp, \
             tc.tile_pool(name="v_t", bufs=3) as vtp, \
             tc.tile_pool(name="p_ps", bufs=4, space="PSUM") as pps:
            for sb in range(NSB):
                hn = hnp.tile([128, 4, D], BF16, name="hn")
                nc.gpsimd.dma_start(
                    out=hn, in_=hflat[sb * 512:(sb + 1) * 512, :]
                    .rearrange("(i p) d -> p i d", p=128))
                hts = htp.tile([128, 4, NDC, 128], BF16, name="hts")
                nc.sync.dma_start(
                    out=hts.rearrange("p i c x -> p (i c) x"),
                    in_=hn.rearrange("p i d -> p (i d)"), transpose=True)
                for w in range(3):
                    pp = pps.tile([128, 512], F32)
                    for dc in range(NDC):
                        nc.tensor.matmul(
                            out=pp,
                            lhsT=wt3[w][:, dc, :],
                            rhs=hts[:, :, dc, :],
                            start=(dc == 0), stop=(dc == NDC - 1))
                    if w < 2:
                        dst = (qt2 if w == 0 else kt2)[:, sb * 512:(sb + 1) * 512]
                        nc.vector.tensor_scalar_add(
                            out=dst, in0=pp, scalar1=bvec[w])
                    else:
                        vt = vtp.tile([128, 512], BF16, name="vt")
                        nc.vector.tensor_scalar_add(out=vt, in0=pp, scalar1=bvec[2])
                        vts = vtp.tile([128, 4, 128], BF16, name="vts")
                        nc.sync.dma_start(out=vts, in_=vt, transpose=True)
                        for i in range(4):
                            kb = sb * 4 + i
                            for h in range(HPC):
                                nc.vector.tensor_copy(
                                    out=va[:, kb, h * 66:h * 66 + 64],
                                    in_=vts[:, i, h * 64:(h + 1) * 64])
                                nc.vector.tensor_copy(
                                    out=va[:, kb, h * 66 + 64:h * 66 + 65],
                                    in_=ones_col)

        # ============ phase 2: attention ==================================
        with tc.tile_pool(name="pt", bufs=6) as ptp, \
             tc.tile_pool(name="stage", bufs=3) as stp, \
             tc.tile_pool(name="osb", bufs=3) as osp, \
             tc.tile_pool(name="sc_ps", bufs=3, space="PSUM") as scp, \
             tc.tile_pool(name="ctx_ps", bufs=4, space="PSUM") as cxp:
            for qb in range(NQB):
                ctx = [[cxp.tile([65, QB], F32, tag="ctx", name=f"ctx{b}{h}")
                        for h in range(HPC)] for b in range(B)]
                # bias^T [k-local, q-sub, k-chunk, q-local] bf16 via one
                # big cast-DMA + one big xbar per head
                bT = [load_biasT(qb, h) for h in range(HPC)]
                for kc in range(NKC):
                    for b in range(B):
                        scs = []
                        for h in range(HPC):
                            sc = scp.tile([128, QB], F32, tag="sc", name="sc")
                            nc.tensor.matmul(
                                out=sc,
                                lhsT=kt2[h * 64:(h + 1) * 64,
                                         b * S + kc * 128:
                                         b * S + (kc + 1) * 128],
                                rhs=qt2[h * 64:(h + 1) * 64,
                                        b * S + qb * QB:
                                        b * S + (qb + 1) * QB],
                                start=True, stop=True,
                                tile_position=(h * 64, 0),
                                skip_group_check=True)
                            scs.append(sc)
                        pts = []
                        for h in range(HPC):
                            # bias add on the (otherwise idle) vector engine
                            nc.vector.tensor_tensor(
                                out=scs[h], in0=scs[h],
                                in1=bT[h][:, :, kc, :],
                                op=mybir.AluOpType.add)
                            pt = ptp.tile([128, QB], BF16, tag="pt", name="pt")
                            nc.scalar.activation(
                                out=pt, in_=scs[h], func=Exp,
                                bias=mb[:, b, kc:kc + 1], scale=SCALE)
                            pts.append(pt)
                        for h in range(HPC):
                            nc.tensor.matmul(
                                out=ctx[b][h],
                                lhsT=va[:, b * NKC + kc,
                                        h * 66:h * 66 + 65],
                                rhs=pts[h],
                                start=(kc == 0), stop=(kc == NKC - 1))
                # ---- epilogue: normalize, transpose to [q, d], store -----
                for b in range(B):
                    stage = stp.tile([128, QB], F32, tag="stage", name="stage")
                    rst = stp.tile([128, QB], F32, tag="rst", name="rst")
                    for h in range(HPC):
                        nc.vector.tensor_copy(
                            out=stage[h * 64:(h + 1) * 64, :],
                            in_=ctx[b][h][0:64, :])
                        # raw denominators at 32-aligned rows 0 / 32
                        nc.vector.tensor_copy(
                            out=rst[32 * h:32 * h + 1, :],
                            in_=ctx[b][h][64:65, :])
                    osb = osp.tile([128, 4, 128], F32, tag="osb", name="osb")
                    for i in range(4):
                        tp = scp.tile([128, 128], F32, tag="sc", name="ep_t")
                        rp = scp.tile([128, 128], F32, tag="sc", name="ep_r")
                        nc.tensor.transpose(
                            out=tp, in_=stage[:, i * 128:(i + 1) * 128],
                            identity=ident)
                        nc.tensor.transpose(
                            out=rp, in_=rst[:, i * 128:(i + 1) * 128],
                            identity=ident)
                        rcp = stp.tile([128, 2], F32, tag="rcp", name="rcp")
                        for h in range(HPC):
                            nc.vector.reciprocal(
                                out=rcp[:, h:h + 1],
                                in_=rp[:, 32 * h:32 * h + 1])
                            nc.vector.tensor_scalar_mul(
                                out=osb[:, i, h * 64:(h + 1) * 64],
                                in0=tp[:, h * 64:(h + 1) * 64],
                                scalar1=rcp[:, h:h + 1])
                    nc.gpsimd.dma_start(
                        out=out[b, qb * QB:(qb + 1) * QB, :]
                        .rearrange("(i p) k -> p i k", p=128),
                        in_=osb)
        btp_cm.__exit__(None, None, None)
        bnp_cm.__exit__(None, None, None)


_CACHE = {}


def _get_program():
    if "nc" not in _CACHE:
        _CACHE["nc"] = _build_program()
    return _CACHE["nc"]


def _shard_inputs(inputs):
    hs = np.ascontiguousarray(np.asarray(inputs["hidden_state"], dtype=np.float32))
    am = np.ascontiguousarray(np.asarray(inputs["attention_mask"], dtype=np.int32))
    ab = np.asarray(inputs["attention_bias"], dtype=np.float32)
    ws = {k: np.asarray(inputs[k], dtype=np.float32) for k in ("Wq", "Wk", "Wv")}
    vb = {k: np.asarray(inputs[k], dtype=np.float32) for k in ("bq", "bk", "bv")}
    in_maps = []
    for c in range(NCORES):
        r0, r1 = c * OC, (c + 1) * OC
        in_maps.append({
            "hidden_state": hs,
            "attention_mask": am,
            "attention_bias": np.ascontiguousarray(ab[0, HPC * c:HPC * (c + 1)]),
            "wq": np.ascontiguousarray(ws["Wq"][r0:r1]),
            "bq": np.ascontiguousarray(vb["bq"][r0:r1]),
            "wk": np.ascontiguousarray(ws["Wk"][r0:r1]),
            "bk": np.ascontiguousarray(vb["bk"][r0:r1]),
            "wv": np.ascontiguousarray(ws["Wv"][r0:r1]),
            "bv": np.ascontiguousarray(vb["bv"][r0:r1]),
        })
    return in_maps


def kernel(**inputs):
    nc = _get_program()
    in_maps = _shard_inputs(inputs)
    res = bass_utils.run_bass_kernel_spmd(
        nc, in_maps, core_ids=list(range(NCORES)))
    parts = [np.asarray(res.results[c]["out"]) for c in range(NCORES)]
    return np.concatenate(parts, axis=-1)


def run_profiled(inputs, trace=True):
    """test.py helper: returns (output, BassKernelResults)."""
    nc = _get_program()
    in_maps = _shard_inputs(inputs)
    res = bass_utils.run_bass_kernel_spmd(
        nc, in_maps, core_ids=list(range(NCORES)), trace=trace)
    parts = [np.asarray(res.results[c]["out"]) for c in range(NCORES)]
    return np.concatenate(parts, axis=-1), res


# revision 18
# speedup vs baseline: 1.7955x; 1.3002x over previous
"""Multi-head self-attention (CogView PB-relax variant) on 8 TRN2 NeuronCores.

Problem: B=2, S=2048, D=1024, H=16 heads, Dh=64.
  q/k/v = hidden @ W{q,k,v}.T + b          (per-head slices)
  scores = (q k^T + attn_bias) / 8 + (1-mask)*(-BIG)
  out    = softmax(scores) @ v             (PB-relax softmax == plain softmax)

Sharding: tensor-parallel over heads. Core c owns heads (2c, 2c+1) for both
batch rows: it reads full hidden, W-row slices [128c:128c+128], bias slice
[h=2c:2c+2], and writes output channels [128c:128(c+1)].

v2 design notes (from v1 profiling):
  - PE-transposes (LDWEIGHTS per 128x128 block + cold-clock streams) were the
    bottleneck; ALL data transposes now ride the 2-byte DMA xbar on bf16:
    one dma_start(transpose=True) turns a [128, N] bf16 SBUF tile into
    N/128 transposed blocks ([128, N/128, 128] out AP).
  - everything PE touches is bf16 (1 cyc/row, FWL weight loads, HAM stays
    warm since the transpose-mode ops are gone).
  - scores computed transposed, tile [k=128, q=512] PSUM: bias^T injected by
    an identity matmul (start=True), then k q^T accumulates (contraction 64,
    both heads concurrently via tile_position row groups).
  - exp on ACT: out = exp(in*0.125 + maskbias[k]) with maskbias a
    per-partition column — the attention mask is free.
  - AV: lhsT = [v | 1] (65 cols), so ctx^T row 64 = masked softmax
    denominator. Epilogue transposes back via PE (small, f32 exact) and
    divides with per-partition reciprocals.
"""

import numpy as np

import concourse.bass as bass
import concourse.mybir as mybir
import concourse.tile as tile
from concourse import bacc, bass_utils
from concourse.masks import make_identity

F32 = mybir.dt.float32
BF16 = mybir.dt.bfloat16
I32 = mybir.dt.int32
Exp = mybir.ActivationFunctionType.Exp

B, S, D = 2, 2048, 1024
NCORES = 8
HPC = 2            # heads per core
OC = HPC * 64      # 128 output channels per core
QB = 512           # q block (free dim of score tiles)
NQB = S // QB      # 4
NKC = S // 128     # 16 k-chunks per batch row
NSB = (B * S) // 512   # 8 token blocks for projections
NDC = D // 128     # 8 contraction chunks

MASK_NEG = -30000.0
SCALE = 0.125


def _build_program():
    nc = bacc.Bacc(
        "TRN2", target_bir_lowering=False, debug=False, num_devices=NCORES
    )
    hidden = nc.dram_tensor("hidden_state", [B, S, D], F32, kind="ExternalInput").ap()
    amask = nc.dram_tensor("attention_mask", [B, S], I32, kind="ExternalInput").ap()
    abias = nc.dram_tensor("attention_bias", [HPC, S, S], F32, kind="ExternalInput").ap()
    wq = nc.dram_tensor("wq", [OC, D], F32, kind="ExternalInput").ap()
    bq = nc.dram_tensor("bq", [OC], F32, kind="ExternalInput").ap()
    wk = nc.dram_tensor("wk", [OC, D], F32, kind="ExternalInput").ap()
    bk = nc.dram_tensor("bk", [OC], F32, kind="ExternalInput").ap()
    wv = nc.dram_tensor("wv", [OC, D], F32, kind="ExternalInput").ap()
    bv = nc.dram_tensor("bv", [OC], F32, kind="ExternalInput").ap()
    out = nc.dram_tensor("out", [B, S, OC], F32, kind="ExternalOutput").ap()

    with tile.TileContext(nc) as tc:
        _attention(tc, out, hidden, amask, abias,
                   [wq, wk, wv], [bq, bk, bv])

    nc.compile()
    return nc


def _attention(tc, out, hidden, amask, abias, ws, bs):
    nc = tc.nc
    hflat = hidden.flatten_outer_dims()          # [4096, 1024]

    with tc.tile_pool(name="singles", bufs=1) as singles:
        ident = singles.tile([128, 128], F32)    # for epilogue PE transposes
        make_identity(nc, ident)
        identb = singles.tile([128, 128], BF16)  # for bias-inject matmuls
        make_identity(nc, identb)

        # --- mask -> additive bias column layout [128, B, NKC] ------------
        mi = singles.tile([128, B, NKC], I32)
        nc.gpsimd.dma_start(out=mi, in_=amask.rearrange("b (c p) -> p b c", p=128))
        mf = singles.tile([128, B, NKC], F32)
        nc.vector.tensor_copy(out=mf, in_=mi)
        mb = singles.tile([128, B, NKC], F32)
        nc.vector.tensor_scalar(
            out=mb, in0=mf, scalar1=-MASK_NEG, scalar2=MASK_NEG,
            op0=mybir.AluOpType.mult, op1=mybir.AluOpType.add,
        )

        # --- projection bias vectors [128, 1] -----------------------------
        bvec = []
        for i, b_ap in enumerate(bs):
            t = singles.tile([128, 1], F32, tag=f"bvec{i}")
            nc.gpsimd.dma_start(out=t, in_=b_ap.rearrange("(p o) -> p o", o=1))
            bvec.append(t)

        ones_col = singles.tile([128, 1], BF16)
        nc.vector.memset(ones_col, 1.0)

        # --- W^T via one merged cast + xbar: wt_all [128, 3*NDC, 128] -----
        with tc.tile_pool(name="w_nat", bufs=1) as wnp:
            wn = wnp.tile([128, 3, D], BF16, name="wn")
            for i, w_ap in enumerate(ws):
                nc.gpsimd.dma_start(out=wn[:, i, :], in_=w_ap)
            wt_all = singles.tile([128, 3 * NDC, 128], BF16, tag="wt")
            nc.sync.dma_start(
                out=wt_all, in_=wn.rearrange("p w d -> p (w d)"), transpose=True)
        wt3 = [wt_all.rearrange("p (w c) x -> p w c x", w=3)[:, i] for i in range(3)]

        # --- persistent activations (bf16) --------------------------------
        qt2 = singles.tile([128, B * S], BF16, tag="qt2")
        kt2 = singles.tile([128, B * S], BF16, tag="kt2")
        va = singles.tile([128, 2 * NKC, 2 * 66], BF16, tag="va")

        # bias prefetch pools live OUTSIDE the phase-1 scope so bias^T
        # streaming overlaps the projections (disjoint SBUF).
        bnp_cm = tc.tile_pool(name="b_nat", bufs=2)
        btp_cm = tc.tile_pool(name="b_t", bufs=3)
        bnp = bnp_cm.__enter__()
        btp = btp_cm.__enter__()

        p1_last_xbar = []   # filled by phase 1; gates bias xbars (order only)

        def load_biasT(qb, h):
            natq = bnp.tile([128, 4, S], BF16, name="natq")
            nc.gpsimd.dma_start(
                out=natq,
                in_=abias[h, qb * QB:(qb + 1) * QB, :]
                .rearrange("(i p) k -> p i k", p=128))
            t = btp.tile([128, 4, NKC, 128], BF16, tag="bT", name=f"bT{h}")
            xb = nc.sync.dma_start(
                out=t.rearrange("p i c x -> p (i c) x"),
                in_=natq.rearrange("p i k -> p (i k)"), transpose=True)
            if p1_last_xbar:
                tile.add_dep_helper(
                    xb.ins, p1_last_xbar[0].ins, sync=False,
                    reason="bias xbar ordered after phase-1 xbars")
            return t

        # ============ phase 1: hidden^T + projections =====================
        with tc.tile_pool(name="h_nat", bufs=2) as hnp, \
             tc.tile_pool(name="h_t", bufs=2) as htp, \
             tc.tile_pool(name="v_t", bufs=3) as vtp, \
             tc.tile_pool(name="p_ps", bufs=4, space="PSUM") as pps:
            for sb in range(NSB):
                hn = hnp.tile([128, 4, D], BF16, name="hn")
                nc.gpsimd.dma_start(
                    out=hn, in_=hflat[sb * 512:(sb + 1) * 512, :]
                    .rearrange("(i p) d -> p i d", p=128))
                hts = htp.tile([128, 4, NDC, 128], BF16, name="hts")
                nc.sync.dma_start(
                    out=hts.rearrange("p i c x -> p (i c) x"),
                    in_=hn.rearrange("p i d -> p (i d)"), transpose=True)
                for w in range(3):
                    pp = pps.tile([128, 512], F32)
                    for dc in range(NDC):
                        nc.tensor.matmul(
                            out=pp,
                            lhsT=wt3[w][:, dc, :],
                            rhs=hts[:, :, dc, :],
                            start=(dc == 0), stop=(dc == NDC - 1))
                    if w < 2:
                        dst = (qt2 if w == 0 else kt2)[:, sb * 512:(sb + 1) * 512]
                        nc.vector.tensor_scalar_add(
                            out=dst, in0=pp, scalar1=bvec[w])
                    else:
                        vt = vtp.tile([128, 512], BF16, name="vt")
                        nc.vector.tensor_scalar_add(out=vt, in0=pp, scalar1=bvec[2])
                        vts = vtp.tile([128, 4, 128], BF16, name="vts")
                        xb = nc.sync.dma_start(out=vts, in_=vt, transpose=True)
                        if sb == NSB - 1:
                            p1_last_xbar.append(xb)
                        for i in range(4):
                            kb = sb * 4 + i
                            for h in range(HPC):
                                nc.vector.tensor_copy(
                                    out=va[:, kb, h * 66:h * 66 + 64],
                                    in_=vts[:, i, h * 64:(h + 1) * 64])
                                nc.vector.tensor_copy(
                                    out=va[:, kb, h * 66 + 64:h * 66 + 65],
                                    in_=ones_col)

        # ============ phase 2: attention ==================================
        with tc.tile_pool(name="pt", bufs=8) as ptp, \
             tc.tile_pool(name="se", bufs=8) as sep, \
             tc.tile_pool(name="stage", bufs=3) as stp, \
             tc.tile_pool(name="osb", bufs=3) as osp, \
             tc.tile_pool(name="sc_ps", bufs=4, space="PSUM") as scp, \
             tc.tile_pool(name="ctx_ps", bufs=4, space="PSUM") as cxp:
            for qb in range(NQB):
                ctx = [[cxp.tile([65, QB], F32, tag="ctx", name=f"ctx{b}{h}")
                        for h in range(HPC)] for b in range(B)]
                # bias^T [k-local, q-sub, k-chunk, q-local] bf16 via one
                # big cast-DMA + one big xbar per head
                bT = [load_biasT(qb, h) for h in range(HPC)]
                for kc in range(NKC):
                    for b in range(B):
                        scs = []
                        for h in range(HPC):
                            sc = scp.tile([128, QB], F32, tag="sc", name="sc")
                            nc.tensor.matmul(
                                out=sc,
                                lhsT=kt2[h * 64:(h + 1) * 64,
                                         b * S + kc * 128:
                                         b * S + (kc + 1) * 128],
                                rhs=qt2[h * 64:(h + 1) * 64,
                                        b * S + qb * QB:
                                        b * S + (qb + 1) * QB],
                                start=True, stop=True,
                                tile_position=(h * 64, 0),
                                skip_group_check=True)
                            scs.append(sc)
                        # bias add on DVE drains PSUM into SBUF (frees the
                        # bank early, decouples ACT from PE)
                        se = sep.tile([128, HPC, QB], F32, tag="se", name="se")
                        for h in range(HPC):
                            nc.vector.tensor_tensor(
                                out=se[:, h, :], in0=scs[h],
                                in1=bT[h][:, :, kc, :],
                                op=mybir.AluOpType.add)
                        pt = ptp.tile([128, HPC, QB], BF16, tag="pt", name="pt")
                        nc.scalar.activation(
                            out=pt.rearrange("p h q -> p (h q)"),
                            in_=se.rearrange("p h q -> p (h q)"), func=Exp,
                            bias=mb[:, b, kc:kc + 1], scale=SCALE)
                        for h in range(HPC):
                            nc.tensor.matmul(
                                out=ctx[b][h],
                                lhsT=va[:, b * NKC + kc,
                                        h * 66:h * 66 + 65],
                                rhs=pt[:, h, :],
                                start=(kc == 0), stop=(kc == NKC - 1))
                # ---- epilogue: normalize, transpose to [q, d], store -----
                for b in range(B):
                    stage = stp.tile([128, QB], F32, tag="stage", name="stage")
                    rst = stp.tile([128, QB], F32, tag="rst", name="rst")
                    for h in range(HPC):
                        nc.vector.tensor_copy(
                            out=stage[h * 64:(h + 1) * 64, :],
                            in_=ctx[b][h][0:64, :])
                        # raw denominators at 32-aligned rows 0 / 32
                        nc.vector.tensor_copy(
                            out=rst[32 * h:32 * h + 1, :],
                            in_=ctx[b][h][64:65, :])
                    osb = osp.tile([128, 4, 128], F32, tag="osb", name="osb")
                    for i in range(4):
                        tp = scp.tile([128, 128], F32, tag="sc", name="ep_t")
                        rp = scp.tile([128, 128], F32, tag="sc", name="ep_r")
                        nc.tensor.transpose(
                            out=tp, in_=stage[:, i * 128:(i + 1) * 128],
                            identity=ident)
                        nc.tensor.transpose(
                            out=rp, in_=rst[:, i * 128:(i + 1) * 128],
                            identity=ident)
                        rcp = stp.tile([128, 2], F32, tag="rcp", name="rcp")
                        for h in range(HPC):
                            nc.vector.reciprocal(
                                out=rcp[:, h:h + 1],
                                in_=rp[:, 32 * h:32 * h + 1])
                            nc.vector.tensor_scalar_mul(
                                out=osb[:, i, h * 64:(h + 1) * 64],
                                in0=tp[:, h * 64:(h + 1) * 64],
                                scalar1=rcp[:, h:h + 1])
                    nc.gpsimd.dma_start(
                        out=out[b, qb * QB:(qb + 1) * QB, :]
                        .rearrange("(i p) k -> p i k", p=128),
                        in_=osb)
        btp_cm.__exit__(None, None, None)
        bnp_cm.__exit__(None, None, None)


_CACHE = {}


def _get_program():
    if "nc" not in _CACHE:
        _CACHE["nc"] = _build_program()
    return _CACHE["nc"]


def _shard_inputs(inputs):
    hs = np.ascontiguousarray(np.asarray(inputs["hidden_state"], dtype=np.float32))
    am = np.ascontiguousarray(np.asarray(inputs["attention_mask"], dtype=np.int32))
    ab = np.asarray(inputs["attention_bias"], dtype=np.float32)
    ws = {k: np.asarray(inputs[k], dtype=np.float32) for k in ("Wq", "Wk", "Wv")}
    vb = {k: np.asarray(inputs[k], dtype=np.float32) for k in ("bq", "bk", "bv")}
    in_maps = []
    for c in range(NCORES):
        r0, r1 = c * OC, (c + 1) * OC
        in_maps.append({
            "hidden_state": hs,
            "attention_mask": am,
            "attention_bias": np.ascontiguousarray(ab[0, HPC * c:HPC * (c + 1)]),
            "wq": np.ascontiguousarray(ws["Wq"][r0:r1]),
            "bq": np.ascontiguousarray(vb["bq"][r0:r1]),
            "wk": np.ascontiguousarray(ws["Wk"][r0:r1]),
            "bk": np.ascontiguousarray(vb["bk"][r0:r1]),
            "wv": np.ascontiguousarray(ws["Wv"][r0:r1]),
            "bv": np.ascontiguousarray(vb["bv"][r0:r1]),
        })
    return in_maps


def kernel(**inputs):
    nc = _get_program()
    in_maps = _shard_inputs(inputs)
    res = bass_utils.run_bass_kernel_spmd(
        nc, in_maps, core_ids=list(range(NCORES)))
    parts = [np.asarray(res.results[c]["out"]) for c in range(NCORES)]
    return np.concatenate(parts, axis=-1)


def run_profiled(inputs, trace=True):
    """test.py helper: returns (output, BassKernelResults)."""
    nc = _get_program()
    in_maps = _shard_inputs(inputs)
    res = bass_utils.run_bass_kernel_spmd(
        nc, in_maps, core_ids=list(range(NCORES)), trace=trace)
    parts = [np.asarray(res.results[c]["out"]) for c in range(NCORES)]
    return np.concatenate(parts, axis=-1), res


# revision 20
# speedup vs baseline: 1.9362x; 1.0783x over previous
"""Multi-head self-attention (CogView PB-relax variant) on 8 TRN2 NeuronCores.

Problem: B=2, S=2048, D=1024, H=16 heads, Dh=64.
  q/k/v = hidden @ W{q,k,v}.T + b          (per-head slices)
  scores = (q k^T + attn_bias) / 8 + (1-mask)*(-BIG)
  out    = softmax(scores) @ v             (PB-relax softmax == plain softmax)

Sharding: tensor-parallel over heads. Core c owns heads (2c, 2c+1) for both
batch rows: it reads full hidden, W-row slices [128c:128c+128], bias slice
[h=2c:2c+2], and writes output channels [128c:128(c+1)].

v2 design notes (from v1 profiling):
  - PE-transposes (LDWEIGHTS per 128x128 block + cold-clock streams) were the
    bottleneck; ALL data transposes now ride the 2-byte DMA xbar on bf16:
    one dma_start(transpose=True) turns a [128, N] bf16 SBUF tile into
    N/128 transposed blocks ([128, N/128, 128] out AP).
  - everything PE touches is bf16 (1 cyc/row, FWL weight loads, HAM stays
    warm since the transpose-mode ops are gone).
  - scores computed transposed, tile [k=128, q=512] PSUM: bias^T injected by
    an identity matmul (start=True), then k q^T accumulates (contraction 64,
    both heads concurrently via tile_position row groups).
  - exp on ACT: out = exp(in*0.125 + maskbias[k]) with maskbias a
    per-partition column — the attention mask is free.
  - AV: lhsT = [v | 1] (65 cols), so ctx^T row 64 = masked softmax
    denominator. Epilogue transposes back via PE (small, f32 exact) and
    divides with per-partition reciprocals.
"""

import numpy as np

import concourse.bass as bass
import concourse.mybir as mybir
import concourse.tile as tile
from concourse import bacc, bass_utils
from concourse.masks import make_identity

F32 = mybir.dt.float32
BF16 = mybir.dt.bfloat16
I32 = mybir.dt.int32
Exp = mybir.ActivationFunctionType.Exp

B, S, D = 2, 2048, 1024
NCORES = 8
HPC = 2            # heads per core
OC = HPC * 64      # 128 output channels per core
QB = 512           # q block (free dim of score tiles)
NQB = S // QB      # 4
NKC = S // 128     # 16 k-chunks per batch row
NSB = (B * S) // 512   # 8 token blocks for projections
NDC = D // 128     # 8 contraction chunks

MASK_NEG = -30000.0
SCALE = 0.125


def _build_program():
    nc = bacc.Bacc(
        "TRN2", target_bir_lowering=False, debug=False, num_devices=NCORES
    )
    hidden = nc.dram_tensor("hidden_state", [B, S, D], F32, kind="ExternalInput").ap()
    amask = nc.dram_tensor("attention_mask", [B, S], I32, kind="ExternalInput").ap()
    abias = nc.dram_tensor("attention_bias", [HPC, S, S], F32, kind="ExternalInput").ap()
    wq = nc.dram_tensor("wq", [OC, D], F32, kind="ExternalInput").ap()
    bq = nc.dram_tensor("bq", [OC], F32, kind="ExternalInput").ap()
    wk = nc.dram_tensor("wk", [OC, D], F32, kind="ExternalInput").ap()
    bk = nc.dram_tensor("bk", [OC], F32, kind="ExternalInput").ap()
    wv = nc.dram_tensor("wv", [OC, D], F32, kind="ExternalInput").ap()
    bv = nc.dram_tensor("bv", [OC], F32, kind="ExternalInput").ap()
    out = nc.dram_tensor("out", [B, S, OC], F32, kind="ExternalOutput").ap()

    with tile.TileContext(nc) as tc:
        _attention(tc, out, hidden, amask, abias,
                   [wq, wk, wv], [bq, bk, bv])

    nc.compile()
    return nc


def _attention(tc, out, hidden, amask, abias, ws, bs):
    nc = tc.nc
    hflat = hidden.flatten_outer_dims()          # [4096, 1024]

    with tc.tile_pool(name="singles", bufs=1) as singles:
        ident = singles.tile([128, 128], F32)    # for epilogue PE transposes
        make_identity(nc, ident)
        identb = singles.tile([128, 128], BF16)  # for bias-inject matmuls
        make_identity(nc, identb)

        # --- mask -> additive bias column layout [128, B, NKC] ------------
        mi = singles.tile([128, B, NKC], I32)
        nc.gpsimd.dma_start(out=mi, in_=amask.rearrange("b (c p) -> p b c", p=128))
        mf = singles.tile([128, B, NKC], F32)
        nc.vector.tensor_copy(out=mf, in_=mi)
        mb = singles.tile([128, B, NKC], F32)
        nc.vector.tensor_scalar(
            out=mb, in0=mf, scalar1=-MASK_NEG, scalar2=MASK_NEG,
            op0=mybir.AluOpType.mult, op1=mybir.AluOpType.add,
        )

        # --- projection bias vectors [128, 1] -----------------------------
        bvec = []
        for i, b_ap in enumerate(bs):
            t = singles.tile([128, 1], F32, tag=f"bvec{i}")
            nc.gpsimd.dma_start(out=t, in_=b_ap.rearrange("(p o) -> p o", o=1))
            bvec.append(t)

        ones_col = singles.tile([128, 1], BF16)
        nc.vector.memset(ones_col, 1.0)

        # --- W^T via one merged cast + xbar: wt_all [128, 3*NDC, 128] -----
        with tc.tile_pool(name="w_nat", bufs=1) as wnp:
            wn = wnp.tile([128, 3, D], BF16, name="wn")
            for i, w_ap in enumerate(ws):
                nc.gpsimd.dma_start(out=wn[:, i, :], in_=w_ap)
            wt_all = singles.tile([128, 3 * NDC, 128], BF16, tag="wt")
            nc.sync.dma_start(
                out=wt_all, in_=wn.rearrange("p w d -> p (w d)"), transpose=True)
        wt3 = [wt_all.rearrange("p (w c) x -> p w c x", w=3)[:, i] for i in range(3)]

        # --- persistent activations (bf16) --------------------------------
        qt2 = singles.tile([128, B * S], BF16, tag="qt2")
        kt2 = singles.tile([128, B * S], BF16, tag="kt2")
        va = singles.tile([128, 2 * NKC, 2 * 66], BF16, tag="va")

        # bias prefetch pools live OUTSIDE the phase-1 scope so bias^T
        # streaming overlaps the projections (disjoint SBUF).
        bnp_cm = tc.tile_pool(name="b_nat", bufs=1)
        btp_cm = tc.tile_pool(name="b_t", bufs=2)
        bnp = bnp_cm.__enter__()
        btp = btp_cm.__enter__()

        p1_last_xbar = []   # filled by phase 1; gates bias xbars (order only)

        def load_biasT(qb):
            # both heads in one cast-DMA + one 2MB xbar:
            # bT2 [k-local, q-sub i, head, k-chunk, q-local]
            natq = bnp.tile([128, 4, HPC, S], BF16, name="natq")
            for h in range(HPC):
                nc.gpsimd.dma_start(
                    out=natq[:, :, h, :],
                    in_=abias[h, qb * QB:(qb + 1) * QB, :]
                    .rearrange("(i p) k -> p i k", p=128))
            t = btp.tile([128, 4, HPC, NKC, 128], BF16, tag="bT", name="bT2")
            xb = nc.sync.dma_start(
                out=t.rearrange("p i h c x -> p (i h c) x"),
                in_=natq.rearrange("p i h k -> p (i h k)"), transpose=True)
            if p1_last_xbar:
                tile.add_dep_helper(
                    xb.ins, p1_last_xbar[0].ins, sync=False,
                    reason="bias xbar ordered after phase-1 xbars")
            return t

        # ============ phase 1: hidden^T + projections =====================
        with tc.tile_pool(name="h_nat", bufs=2) as hnp, \
             tc.tile_pool(name="h_t", bufs=2) as htp, \
             tc.tile_pool(name="v_t", bufs=3) as vtp, \
             tc.tile_pool(name="p_ps", bufs=4, space="PSUM") as pps:
            pend_vt2 = []
            for sb in range(NSB):
                hn = hnp.tile([128, 4, D], BF16, name="hn")
                nc.gpsimd.dma_start(
                    out=hn, in_=hflat[sb * 512:(sb + 1) * 512, :]
                    .rearrange("(i p) d -> p i d", p=128))
                hts = htp.tile([128, 4, NDC, 128], BF16, name="hts")
                nc.sync.dma_start(
                    out=hts.rearrange("p i c x -> p (i c) x"),
                    in_=hn.rearrange("p i d -> p (i d)"), transpose=True)
                for w in range(3):
                    pp = pps.tile([128, 512], F32)
                    for dc in range(NDC):
                        nc.tensor.matmul(
                            out=pp,
                            lhsT=wt3[w][:, dc, :],
                            rhs=hts[:, :, dc, :],
                            start=(dc == 0), stop=(dc == NDC - 1))
                    if w < 2:
                        dst = (qt2 if w == 0 else kt2)[:, sb * 512:(sb + 1) * 512]
                        nc.scalar.activation(
                            out=dst, in_=pp,
                            func=mybir.ActivationFunctionType.Identity,
                            bias=bvec[w])
                    else:
                        if sb % 2 == 0:
                            vt2 = vtp.tile([128, 2, 512], BF16, name="vt2")
                            pend_vt2.append(vt2)
                        else:
                            vt2 = pend_vt2[-1]
                        nc.vector.tensor_scalar_add(
                            out=vt2[:, sb % 2, :], in0=pp, scalar1=bvec[2])
                        if sb % 2 == 1:
                            vts = vtp.tile([128, 8, 128], BF16, name="vts")
                            xb = nc.sync.dma_start(
                                out=vts, in_=vt2.rearrange("p j q -> p (j q)"),
                                transpose=True)
                            if sb == NSB - 1:
                                p1_last_xbar.append(xb)
                            for j in range(8):
                                kb = (sb - 1) * 4 + j
                                for h in range(HPC):
                                    nc.vector.tensor_copy(
                                        out=va[:, kb, h * 66:h * 66 + 64],
                                        in_=vts[:, j, h * 64:(h + 1) * 64])
                                    nc.vector.tensor_copy(
                                        out=va[:, kb, h * 66 + 64:h * 66 + 65],
                                        in_=ones_col)

        # ============ phase 2: attention ==================================
        with tc.tile_pool(name="pt", bufs=8) as ptp, \
             tc.tile_pool(name="se", bufs=8) as sep, \
             tc.tile_pool(name="stage", bufs=3) as stp, \
             tc.tile_pool(name="osb", bufs=3) as osp, \
             tc.tile_pool(name="sc_ps", bufs=4, space="PSUM") as scp, \
             tc.tile_pool(name="ctx_ps", bufs=4, space="PSUM") as cxp:
            for qb in range(NQB):
                ctx = [[cxp.tile([65, QB], F32, tag="ctx", name=f"ctx{b}{h}")
                        for h in range(HPC)] for b in range(B)]
                bT2 = load_biasT(qb)
                for kc in range(NKC):
                    for b in range(B):
                        scs = []
                        for h in range(HPC):
                            sc = scp.tile([128, QB], F32, tag="sc", name="sc")
                            nc.tensor.matmul(
                                out=sc,
                                lhsT=kt2[h * 64:(h + 1) * 64,
                                         b * S + kc * 128:
                                         b * S + (kc + 1) * 128],
                                rhs=qt2[h * 64:(h + 1) * 64,
                                        b * S + qb * QB:
                                        b * S + (qb + 1) * QB],
                                start=True, stop=True,
                                tile_position=(h * 64, 0),
                                skip_group_check=True)
                            scs.append(sc)
                        # bias add on DVE drains PSUM into SBUF (frees the
                        # bank early, decouples ACT from PE)
                        se = sep.tile([128, HPC, QB], F32, tag="se", name="se")
                        for h in range(HPC):
                            nc.vector.tensor_tensor(
                                out=se[:, h, :], in0=scs[h],
                                in1=bT2[:, :, h, kc, :],
                                op=mybir.AluOpType.add)
                        pt = ptp.tile([128, HPC, QB], BF16, tag="pt", name="pt")
                        nc.scalar.activation(
                            out=pt.rearrange("p h q -> p (h q)"),
                            in_=se.rearrange("p h q -> p (h q)"), func=Exp,
                            bias=mb[:, b, kc:kc + 1], scale=SCALE)
                        for h in range(HPC):
                            nc.tensor.matmul(
                                out=ctx[b][h],
                                lhsT=va[:, b * NKC + kc,
                                        h * 66:h * 66 + 65],
                                rhs=pt[:, h, :],
                                start=(kc == 0), stop=(kc == NKC - 1))
                # ---- epilogue: normalize, transpose to [q, d], store -----
                for b in range(B):
                    stage = stp.tile([128, QB], F32, tag="stage", name="stage")
                    rst = stp.tile([128, QB], F32, tag="rst", name="rst")
                    for h in range(HPC):
                        nc.vector.tensor_copy(
                            out=stage[h * 64:(h + 1) * 64, :],
                            in_=ctx[b][h][0:64, :])
                        # raw denominators at 32-aligned rows 0 / 32
                        nc.vector.tensor_copy(
                            out=rst[32 * h:32 * h + 1, :],
                            in_=ctx[b][h][64:65, :])
                    osb = osp.tile([128, 4, 128], F32, tag="osb", name="osb")
                    for i in range(4):
                        tp = scp.tile([128, 128], F32, tag="sc", name="ep_t")
                        rp = scp.tile([128, 128], F32, tag="sc", name="ep_r")
                        nc.tensor.transpose(
                            out=tp, in_=stage[:, i * 128:(i + 1) * 128],
                            identity=ident)
                        nc.tensor.transpose(
                            out=rp, in_=rst[:, i * 128:(i + 1) * 128],
                            identity=ident)
                        rcp = stp.tile([128, 2], F32, tag="rcp", name="rcp")
                        for h in range(HPC):
                            nc.vector.reciprocal(
                                out=rcp[:, h:h + 1],
                                in_=rp[:, 32 * h:32 * h + 1])
                            nc.vector.tensor_scalar_mul(
                                out=osb[:, i, h * 64:(h + 1) * 64],
                                in0=tp[:, h * 64:(h + 1) * 64],
                                scalar1=rcp[:, h:h + 1])
                    nc.gpsimd.dma_start(
                        out=out[b, qb * QB:(qb + 1) * QB, :]
                        .rearrange("(i p) k -> p i k", p=128),
                        in_=osb)
        btp_cm.__exit__(None, None, None)
        bnp_cm.__exit__(None, None, None)


_CACHE = {}


def _get_program():
    if "nc" not in _CACHE:
        _CACHE["nc"] = _build_program()
    return _CACHE["nc"]


def _shard_inputs(inputs):
    hs = np.ascontiguousarray(np.asarray(inputs["hidden_state"], dtype=np.float32))
    am = np.ascontiguousarray(np.asarray(inputs["attention_mask"], dtype=np.int32))
    ab = np.asarray(inputs["attention_bias"], dtype=np.float32)
    ws = {k: np.asarray(inputs[k], dtype=np.float32) for k in ("Wq", "Wk", "Wv")}
    vb = {k: np.asarray(inputs[k], dtype=np.float32) for k in ("bq", "bk", "bv")}
    in_maps = []
    for c in range(NCORES):
        r0, r1 = c * OC, (c + 1) * OC
        in_maps.append({
            "hidden_state": hs,
            "attention_mask": am,
            "attention_bias": np.ascontiguousarray(ab[0, HPC * c:HPC * (c + 1)]),
            "wq": np.ascontiguousarray(ws["Wq"][r0:r1]),
            "bq": np.ascontiguousarray(vb["bq"][r0:r1]),
            "wk": np.ascontiguousarray(ws["Wk"][r0:r1]),
            "bk": np.ascontiguousarray(vb["bk"][r0:r1]),
            "wv": np.ascontiguousarray(ws["Wv"][r0:r1]),
            "bv": np.ascontiguousarray(vb["bv"][r0:r1]),
        })
    return in_maps


def kernel(**inputs):
    nc = _get_program()
    in_maps = _shard_inputs(inputs)
    res = bass_utils.run_bass_kernel_spmd(
        nc, in_maps, core_ids=list(range(NCORES)))
    parts = [np.asarray(res.results[c]["out"]) for c in range(NCORES)]
    return np.concatenate(parts, axis=-1)


def run_profiled(inputs, trace=True):
    """test.py helper: returns (output, BassKernelResults)."""
    nc = _get_program()
    in_maps = _shard_inputs(inputs)
    res = bass_utils.run_bass_kernel_spmd(
        nc, in_maps, core_ids=list(range(NCORES)), trace=trace)
    parts = [np.asarray(res.results[c]["out"]) for c in range(NCORES)]
    return np.concatenate(parts, axis=-1), res
